# revision 18
# baseline (speedup 1.0000x reference)
"""Trainium2 Bass kernel for nn_MessagePassingLayer (gnn_message_passing).

Math: out[b,i,f] = softplus( sum_j G(d_ij)[f] * fb[b,j,f] ), where
  d_ij = ||r_i - r_j||  and  G(d) = softplus(W2^T softplus(w1*d + b1) + b2).

Device strategy (8 cores, data-parallel over B):
  G depends on the scalar d only, so at runtime we fit (on host, float64)
  an exponential-sum surrogate  G(d)[f] ~= sum_k C[k,f] * exp(gam[k]*d)
  (K=32 terms, abs err ~1e-6, well under fp32 noise).  Per core (2 batches):
    1. d matrix via one K=5 matmul per 128-row chunk using the augmented
       factorization s = |ri|^2 + |rj|^2 - 2 ri.rj, then d = exp(0.5*ln(s))
       on ACT (keeps everything in the exp/ln table set; no sqrt set load).
    2. For each group of 4 j-rows: one K=32 selector-matmul broadcasts the
       4 d-rows into a [4x32, 256] block of gam[k]*d values; ACT exp gives
       the basis; one K=128 matmul with an f-scaled coefficient block
       (C[k,f]*fb[j,f], built by DVE from a broadcast DMA of f) both applies
       G and reduces over the 4 j's, PSUM-accumulating over all 64 groups.
    3. softplus on the [64,256] accumulator (exp then ln(1+x)), PE-transpose,
       DMA out.
"""

import numpy as np

import concourse.bass as bass
import concourse.bacc as bacc
import concourse.tile as tile
from concourse import mybir
from concourse.bass_utils import run_bass_kernel_spmd

AF = mybir.ActivationFunctionType
F32 = mybir.dt.float32

B, N, F, H = 16, 256, 64, 128
NCORES = 8
NB = B // NCORES          # batches per core
PACK = 8                  # j's packed per 128-partition basis block
KB = 128 // PACK          # exponential-basis size
NPOS = 4                  # growing exponentials (rest decay, cheb-spaced)
SMAX, SPOS = 4.0, 0.5
DMAX = float(np.sqrt(3.0)) * 1.002
NGRP = N // PACK          # groups of PACK j's per batch
NBLK = NGRP // 4          # blocks of 4 groups
TPB = 32 // PACK          # groups per 32-row chunk of the d matrix


def basis_gammas() -> np.ndarray:
    nneg = KB - NPOS
    x = (1.0 - np.cos(np.pi * np.arange(nneg) / (nneg - 1))) / 2.0
    neg = -SMAX * x
    pos = SPOS * np.arange(1, NPOS + 1, dtype=np.float64)
    return np.concatenate([neg, pos])


def fit_coeffs(w1, b1, W2, b2) -> np.ndarray:
    """Least-squares fit of C [KB, F] s.t. exp(d*gam) @ C ~= G(d)."""
    gam = basis_gammas()
    dgrid = np.concatenate([[0.0], np.linspace(0.0, DMAX, 6001)[1:]])
    A = np.exp(np.outer(dgrid, gam))
    h = np.logaddexp(np.outer(dgrid, w1.astype(np.float64)) + b1.astype(np.float64), 0.0)
    tgt = np.logaddexp(h @ W2.astype(np.float64) + b2.astype(np.float64), 0.0)
    AtA = A.T @ A + 1e-10 * np.eye(KB)
    C = np.linalg.solve(AtA, A.T @ tgt)
    return C  # float64 [KB, F]


def make_ksel(gam: np.ndarray) -> np.ndarray:
    """Selector weights [128, TPB*128]: for matmul t, column m=KB*q+k' picks
    gam[k'] * d_row(j'=PACK*t+q).  Row p encodes j' = p%32 (same for each
    32-row group so any 32-aligned slice works)."""
    g32 = gam.astype(np.float32)
    ksel = np.zeros((128, TPB * 128), np.float32)
    for p in range(128):
        jp = p % 32
        t, q = jp // PACK, jp % PACK
        ksel[p, 128 * t + KB * q: 128 * t + KB * q + KB] = g32
    return ksel


def make_cstack(C: np.ndarray) -> np.ndarray:
    """[128, 4F]: cstack[KB*q+k', 64*g4+f] = C[k', f]."""
    return np.tile(C.astype(np.float32), (PACK, 4))


def _pin_act_table(arch: str):
    """Make Exp/Ln resolve only to the combined natural_log_exp_and_others
    set, so the whole kernel needs a single ACT table load (the default
    first-match picks exp_and_others for Exp and natural_log for Ln, which
    thrashes ~2.7us per switch).  Mutates the cached dict in place, which
    preserves act_func_set_id indices."""
    from concourse.hw_specs import get_activation_tables
    tabs = get_activation_tables(arch)
    for name, fns in tabs.items():
        if name != "natural_log_exp_and_others":
            fns.discard(AF.Exp)
            fns.discard(AF.Ln)


BASIS_DMA = False  # False: selector-matmul basis (PE) instead of DMA bcast


def build_nc(reps: int = 1):
    """One-core program processing NB batches; SPMD across 8 cores."""
    nc = bacc.Bacc("TRN2", target_bir_lowering=False, debug=False)
    _pin_act_table(nc.m.arch)

    r_h = nc.dram_tensor("r", [NB, N, 3], F32, kind="ExternalInput")
    f_h = nc.dram_tensor("f", [NB, N, F], F32, kind="ExternalInput")
    fsel_h = nc.dram_tensor("fsel", [16, 256], F32, kind="ExternalInput")
    ksel_h = nc.dram_tensor("ksel", [128, TPB * 128], F32, kind="ExternalInput")
    cstk_h = nc.dram_tensor("cstk", [128, 4 * F], F32, kind="ExternalInput")
    ident_h = nc.dram_tensor("ident", [64, 64], F32, kind="ExternalInput")
    gamv_h = nc.dram_tensor("gamv", [128, 1], F32, kind="ExternalInput")
    out_h = nc.dram_tensor("out", [NB, N, F], F32, kind="ExternalOutput")

    with tile.TileContext(nc) as tc:
        with (
            tc.tile_pool(name="statics", bufs=1) as statics,
            tc.tile_pool(name="prep", bufs=2) as prep,
            tc.tile_pool(name="dpool", bufs=4) as dpool,
            tc.tile_pool(name="pbas", bufs=3) as pbas,
            tc.tile_pool(name="cfp", bufs=3) as cfp,
            tc.tile_pool(name="outp", bufs=4) as outp,
            tc.tile_pool(name="psP", bufs=2, space="PSUM") as psP,
            tc.tile_pool(name="psF", bufs=2, space="PSUM") as psF,
            tc.tile_pool(name="psM", bufs=2, space="PSUM") as psM,
        ):
            ksel_sb = statics.tile([128, TPB * 128], F32)
            nc.sync.dma_start(out=ksel_sb, in_=ksel_h.ap())
            cstk_sb = statics.tile([128, 4 * F], F32)
            nc.sync.dma_start(out=cstk_sb, in_=cstk_h.ap())
            ident_sb = statics.tile([64, 64], F32)
            nc.sync.dma_start(out=ident_sb, in_=ident_h.ap())
            ones3 = statics.tile([3, 1], F32)
            nc.vector.memset(ones3, 1.0)
            gamv_sb = statics.tile([128, 1], F32)
            nc.sync.dma_start(out=gamv_sb, in_=gamv_h.ap())
            ones_row = statics.tile([1, N], F32)
            nc.vector.memset(ones_row, 1.0)
            fsel_sb = statics.tile([16, 256], F32)
            nc.sync.dma_start(out=fsel_sb, in_=fsel_h.ap())

            def prologue(b):
                # d[j,i] for batch b as two [128, N] chunks (rows = j).
                # s = -2*r.r^T + n_i + n_j built by three accumulating
                # matmuls (K=3 product + two K=1 rank-1 adds); d = e^{ln(s)/2}.
                rt = prep.tile([3, N], F32)
                nc.sync.dma_start(
                    out=rt,
                    in_=bass.AP(tensor=r_h, offset=b * N * 3,
                                ap=[[1, 3], [3, N]]),
                )
                sq = prep.tile([3, N], F32)
                nc.vector.tensor_mul(sq, rt, rt)
                m2rt = prep.tile([3, N], F32)
                nc.vector.tensor_scalar_mul(m2rt, rt, -2.0)
                n_ps = psM.tile([1, N], F32, tag="m")
                nc.tensor.matmul(out=n_ps, lhsT=ones3, rhs=sq,
                                 start=True, stop=True)
                n_sb = prep.tile([1, N], F32)
                nc.vector.tensor_copy(n_sb, n_ps)
                f_sb = prep.tile([16, 16 * F], F32)
                nc.sync.dma_start(
                    out=f_sb,
                    in_=bass.AP(tensor=f_h, offset=b * N * F,
                                ap=[[F, 16], [16 * F, 16], [1, F]]),
                )

                d_sb = []
                for c in range(2):
                    s_ps = psM.tile([128, N], F32, tag="m")
                    sl = slice(128 * c, 128 * (c + 1))
                    nc.tensor.matmul(out=s_ps, lhsT=m2rt[:, sl], rhs=rt,
                                     start=True, stop=False)
                    nc.tensor.matmul(out=s_ps, lhsT=n_sb[:, sl], rhs=ones_row,
                                     start=False, stop=False)
                    nc.tensor.matmul(out=s_ps, lhsT=ones_row[:, sl], rhs=n_sb,
                                     start=False, stop=True)
                    smax = prep.tile([128, N], F32)
                    nc.vector.tensor_scalar_max(smax, s_ps, 1e-30)
                    lns = prep.tile([128, N], F32)
                    nc.scalar.activation(lns, smax, AF.Ln)
                    dc = dpool.tile([128, N], F32)
                    nc.scalar.activation(dc, lns, AF.Exp, scale=0.5)
                    d_sb.append(dc)
                return d_sb, f_sb

            def main(b, d_sb, f_sb):
                # d-row broadcast via SBUF->SBUF DMA (replicates each of the
                # 8 j-rows KB times down the partitions); gam multiply rides
                # on the ACT exp via per-partition scale.  One K=128 matmul
                # per group applies C*f and reduces over its PACK j's,
                # accumulating in PSUM over all groups.
                fbar = psF.tile([64, N], F32)
                GB = 8 if BASIS_DMA else 4  # groups per Ps tile
                for sb_blk in range(NGRP // GB):
                    if BASIS_DMA:
                        dbc = pbas.tile([128, GB * N], F32, tag="dbc")
                        for gg in range(GB):
                            g = sb_blk * GB + gg
                            c = g // (NGRP // 2)
                            p0 = PACK * (g % (NGRP // 2))
                            src_ap = d_sb[c][p0: p0 + PACK, :]
                            eng = (nc.sync, nc.gpsimd, nc.scalar)[gg % 3]
                            eng.dma_start(
                                out=dbc[:, N * gg: N * (gg + 1)],
                                in_=bass.AP(tensor=src_ap.tensor,
                                            offset=src_ap.offset,
                                            ap=[src_ap.ap[0], [0, KB], [1, N]]),
                            )
                        Ps = pbas.tile([128, GB * N], F32, tag="ps")
                        nc.scalar.activation(Ps, dbc, AF.Exp, scale=gamv_sb)
                    else:
                        Pp = psP.tile([128, GB * N], F32)
                        for gg in range(GB):
                            g = sb_blk * GB + gg
                            c = g // (NGRP // 2)
                            G = (g % (NGRP // 2)) // TPB
                            t = g % TPB
                            nc.tensor.matmul(
                                out=Pp[:, N * gg: N * (gg + 1)],
                                lhsT=ksel_sb[32 * G: 32 * (G + 1),
                                             128 * t: 128 * (t + 1)],
                                rhs=d_sb[c][32 * G: 32 * (G + 1), :],
                                start=True, stop=True,
                                tile_position=(32 * G, 0),
                            )
                        Ps = pbas.tile([128, GB * N], F32, tag="ps")
                        nc.scalar.activation(Ps, Pp, AF.Exp)

                    for fb4 in range(GB // 4):
                        blk = sb_blk * (GB // 4) + fb4
                        fb_ps = psM.tile([128, 4 * F], F32, tag="m")
                        for g4 in range(4):
                            g = blk * 4 + g4
                            t, jblk = g % 2, g // 2
                            nc.tensor.matmul(
                                out=fb_ps[:, F * g4: F * (g4 + 1)],
                                lhsT=fsel_sb[:, 128 * t: 128 * (t + 1)],
                                rhs=f_sb[:, F * jblk: F * (jblk + 1)],
                                start=True, stop=True,
                            )
                        cf = cfp.tile([128, 4 * F], F32)
                        nc.vector.tensor_mul(cf, cstk_sb, fb_ps)
                        for g4 in range(4):
                            g = blk * 4 + g4
                            gg = g - sb_blk * GB
                            nc.tensor.matmul(
                                out=fbar,
                                lhsT=cf[:, F * g4: F * (g4 + 1)],
                                rhs=Ps[:, N * gg: N * (gg + 1)],
                                start=(g == 0), stop=(g == NGRP - 1),
                            )
                return fbar

            def epilogue(b, fbar):
                # stable softplus x + ln(1+e^-x) (f_bar > 0), transpose, store
                esb = outp.tile([64, N], F32)
                nc.scalar.activation(esb, fbar, AF.Exp, scale=-1.0)
                lsb = outp.tile([64, N], F32)
                nc.scalar.activation(lsb, esb, AF.Ln, bias=1.0)
                osb = outp.tile([64, N], F32)
                nc.vector.tensor_add(osb, fbar, lsb)
                t_sb = outp.tile([128, 2 * F], F32)
                for hh in range(2):
                    t_ps = psM.tile([128, 64], F32, tag="m")
                    nc.tensor.transpose(t_ps, osb[:, 128 * hh: 128 * (hh + 1)],
                                        ident_sb)
                    nc.vector.tensor_copy(t_sb[:, F * hh: F * (hh + 1)], t_ps)
                nc.sync.dma_start(
                    out=bass.AP(tensor=out_h, offset=b * N * F,
                                ap=[[F, 128], [128 * F, 2], [1, F]]),
                    in_=t_sb,
                )

            def emit_body():
                pro = [prologue(b) for b in range(NB)]
                fbars = [main(b, *pro[b]) for b in range(NB)]
                for b in range(NB):
                    epilogue(b, fbars[b])

            if reps > 1:
                with tc.For_i(0, reps, 1):
                    emit_body()
            else:
                emit_body()

    nc.compile()
    return nc


_CACHE: dict = {}


def _get_nc(reps: int = 1):
    if reps not in _CACHE:
        _CACHE[reps] = build_nc(reps)
    return _CACHE[reps]


def make_fsel() -> np.ndarray:
    """[16, 256]: fsel[j', 128t + 16q + k'] = (j' == 8t + q), so a K=16
    matmul against 16 f-rows replicates row 8t+q across the k' partitions."""
    fsel = np.zeros((16, 256), np.float32)
    for t in range(2):
        for q in range(PACK):
            fsel[8 * t + q, 128 * t + KB * q: 128 * t + KB * q + KB] = 1.0
    return fsel


def make_inputs_per_core(r_batch, f_batch, w1, b1, W2, b2):
    r_batch = np.ascontiguousarray(np.asarray(r_batch, np.float32))
    f_batch = np.ascontiguousarray(np.asarray(f_batch, np.float32))
    C = fit_coeffs(np.asarray(w1, np.float64), np.asarray(b1, np.float64),
                   np.asarray(W2, np.float64), np.asarray(b2, np.float64))
    gam = basis_gammas()
    ksel = make_ksel(gam)
    cstk = make_cstack(C)
    ident = np.eye(64, dtype=np.float32)
    gamv = np.ascontiguousarray(np.tile(gam.astype(np.float32), PACK)[:, None])
    in_maps = []
    for c in range(NCORES):
        in_maps.append({
            "r": r_batch[NB * c: NB * (c + 1)],
            "f": f_batch[NB * c: NB * (c + 1)],
            "fsel": make_fsel(),
            "ksel": ksel,
            "cstk": cstk,
            "ident": ident,
            "gamv": gamv,
        })
    return in_maps


def kernel(r_batch, f_batch, w1, b1, W2, b2):
    in_maps = make_inputs_per_core(r_batch, f_batch, w1, b1, W2, b2)
    nc = _get_nc(1)
    res = run_bass_kernel_spmd(nc, in_maps, list(range(NCORES)))
    out = np.concatenate([res.results[c]["out"] for c in range(NCORES)], axis=0)
    return out.astype(np.float32)


# revision 19
# speedup vs baseline: 1.2054x; 1.2054x over previous
"""Trainium2 Bass kernel for nn_MessagePassingLayer (gnn_message_passing).

Math: out[b,i,f] = softplus( sum_j G(d_ij)[f] * fb[b,j,f] ), where
  d_ij = ||r_i - r_j||  and  G(d) = softplus(W2^T softplus(w1*d + b1) + b2).

Device strategy (8 cores, data-parallel over B):
  G depends on the scalar d only, so at runtime we fit (on host, float64)
  an exponential-sum surrogate  G(d)[f] ~= sum_k C[k,f] * exp(gam[k]*d)
  (K=32 terms, abs err ~1e-6, well under fp32 noise).  Per core (2 batches):
    1. d matrix via one K=5 matmul per 128-row chunk using the augmented
       factorization s = |ri|^2 + |rj|^2 - 2 ri.rj, then d = exp(0.5*ln(s))
       on ACT (keeps everything in the exp/ln table set; no sqrt set load).
    2. For each group of 4 j-rows: one K=32 selector-matmul broadcasts the
       4 d-rows into a [4x32, 256] block of gam[k]*d values; ACT exp gives
       the basis; one K=128 matmul with an f-scaled coefficient block
       (C[k,f]*fb[j,f], built by DVE from a broadcast DMA of f) both applies
       G and reduces over the 4 j's, PSUM-accumulating over all 64 groups.
    3. softplus on the [64,256] accumulator (exp then ln(1+x)), PE-transpose,
       DMA out.
"""

import numpy as np

import concourse.bass as bass
import concourse.bacc as bacc
import concourse.tile as tile
from concourse import mybir
from concourse.bass_utils import run_bass_kernel_spmd

AF = mybir.ActivationFunctionType
F32 = mybir.dt.float32

B, N, F, H = 16, 256, 64, 128
NCORES = 8
NB = B // NCORES          # batches per core
PACK = 8                  # j's packed per 128-partition basis block
KB = 128 // PACK          # exponential-basis size
NPOS = 4                  # growing exponentials (rest decay, cheb-spaced)
SMAX, SPOS = 4.0, 0.5
DMAX = float(np.sqrt(3.0)) * 1.002
NGRP = N // PACK          # groups of PACK j's per batch
NBLK = NGRP // 4          # blocks of 4 groups
TPB = 32 // PACK          # groups per 32-row chunk of the d matrix


def basis_gammas() -> np.ndarray:
    nneg = KB - NPOS
    x = (1.0 - np.cos(np.pi * np.arange(nneg) / (nneg - 1))) / 2.0
    neg = -SMAX * x
    pos = SPOS * np.arange(1, NPOS + 1, dtype=np.float64)
    return np.concatenate([neg, pos])


def fit_coeffs(w1, b1, W2, b2) -> np.ndarray:
    """Least-squares fit of C [KB, F] s.t. exp(d*gam) @ C ~= G(d)."""
    gam = basis_gammas()
    dgrid = np.concatenate([[0.0], np.linspace(0.0, DMAX, 6001)[1:]])
    A = np.exp(np.outer(dgrid, gam))
    h = np.logaddexp(np.outer(dgrid, w1.astype(np.float64)) + b1.astype(np.float64), 0.0)
    tgt = np.logaddexp(h @ W2.astype(np.float64) + b2.astype(np.float64), 0.0)
    AtA = A.T @ A + 1e-10 * np.eye(KB)
    C = np.linalg.solve(AtA, A.T @ tgt)
    return C  # float64 [KB, F]


def make_ksel(gam: np.ndarray) -> np.ndarray:
    """Selector weights [128, TPB*128]: for matmul t, column m=KB*q+k' picks
    gam[k'] * d_row(j'=PACK*t+q).  Row p encodes j' = p%32 (same for each
    32-row group so any 32-aligned slice works)."""
    g32 = gam.astype(np.float32)
    ksel = np.zeros((128, TPB * 128), np.float32)
    for p in range(128):
        jp = p % 32
        t, q = jp // PACK, jp % PACK
        ksel[p, 128 * t + KB * q: 128 * t + KB * q + KB] = g32
    return ksel


def make_cstack(C: np.ndarray) -> np.ndarray:
    """[128, 4F]: cstack[KB*q+k', 64*g4+f] = C[k', f]."""
    return np.tile(C.astype(np.float32), (PACK, 4))


def _pin_act_table(arch: str):
    """Make Exp/Ln resolve only to the combined natural_log_exp_and_others
    set, so the whole kernel needs a single ACT table load (the default
    first-match picks exp_and_others for Exp and natural_log for Ln, which
    thrashes ~2.7us per switch).  Mutates the cached dict in place, which
    preserves act_func_set_id indices."""
    from concourse.hw_specs import get_activation_tables
    tabs = get_activation_tables(arch)
    for name, fns in tabs.items():
        if name != "natural_log_exp_and_others":
            fns.discard(AF.Exp)
            fns.discard(AF.Ln)


BASIS_DMA = False  # False: selector-matmul basis (PE) instead of DMA bcast


def build_nc(reps: int = 1):
    """One-core program processing NB batches; SPMD across 8 cores."""
    nc = bacc.Bacc("TRN2", target_bir_lowering=False, debug=False)
    _pin_act_table(nc.m.arch)

    r_h = nc.dram_tensor("r", [NB, N, 3], F32, kind="ExternalInput")
    f_h = nc.dram_tensor("f", [NB, N, F], F32, kind="ExternalInput")
    fsel_h = nc.dram_tensor("fsel", [16, 256], F32, kind="ExternalInput")
    ksel_h = nc.dram_tensor("ksel", [128, TPB * 128], F32, kind="ExternalInput")
    cstk_h = nc.dram_tensor("cstk", [128, 4 * F], F32, kind="ExternalInput")
    ident_h = nc.dram_tensor("ident", [64, 64], F32, kind="ExternalInput")
    gamv_h = nc.dram_tensor("gamv", [128, 1], F32, kind="ExternalInput")
    out_h = nc.dram_tensor("out", [NB, N, F], F32, kind="ExternalOutput")

    with tile.TileContext(nc) as tc:
        with (
            tc.tile_pool(name="statics", bufs=1) as statics,
            tc.tile_pool(name="prep", bufs=2) as prep,
            tc.tile_pool(name="dpool", bufs=4) as dpool,
            tc.tile_pool(name="pbas", bufs=3) as pbas,
            tc.tile_pool(name="cfp", bufs=3) as cfp,
            tc.tile_pool(name="outp", bufs=4) as outp,
            tc.tile_pool(name="psP", bufs=2, space="PSUM") as psP,
            tc.tile_pool(name="psF", bufs=2, space="PSUM") as psF,
            tc.tile_pool(name="psM", bufs=2, space="PSUM") as psM,
        ):
            ksel_sb = statics.tile([128, TPB * 128], F32)
            nc.sync.dma_start(out=ksel_sb, in_=ksel_h.ap())
            cstk_sb = statics.tile([128, 4 * F], F32)
            nc.sync.dma_start(out=cstk_sb, in_=cstk_h.ap())
            ident_sb = statics.tile([64, 64], F32)
            nc.sync.dma_start(out=ident_sb, in_=ident_h.ap())
            ones3 = statics.tile([3, 1], F32)
            nc.vector.memset(ones3, 1.0)
            gamv_sb = statics.tile([128, 1], F32)
            nc.sync.dma_start(out=gamv_sb, in_=gamv_h.ap())
            ones_row = statics.tile([1, N], F32)
            nc.vector.memset(ones_row, 1.0)
            fsel_sb = statics.tile([16, 256], F32)
            nc.sync.dma_start(out=fsel_sb, in_=fsel_h.ap())

            def prologue(b):
                # d[j,i] for batch b as two [128, N] chunks (rows = j).
                # s = -2*r.r^T + n_i + n_j built by three accumulating
                # matmuls (K=3 product + two K=1 rank-1 adds); d = e^{ln(s)/2}.
                rt = prep.tile([3, N], F32)
                nc.sync.dma_start(
                    out=rt,
                    in_=bass.AP(tensor=r_h, offset=b * N * 3,
                                ap=[[1, 3], [3, N]]),
                )
                sq = prep.tile([3, N], F32)
                nc.vector.tensor_mul(sq, rt, rt)
                m2rt = prep.tile([3, N], F32)
                nc.vector.tensor_scalar_mul(m2rt, rt, -2.0)
                n_ps = psM.tile([1, N], F32, tag="m")
                nc.tensor.matmul(out=n_ps, lhsT=ones3, rhs=sq,
                                 start=True, stop=True)
                n_sb = prep.tile([1, N], F32)
                nc.vector.tensor_copy(n_sb, n_ps)
                f_sb = prep.tile([16, 16 * F], F32)
                nc.sync.dma_start(
                    out=f_sb,
                    in_=bass.AP(tensor=f_h, offset=b * N * F,
                                ap=[[F, 16], [16 * F, 16], [1, F]]),
                )

                d_sb = []
                for c in range(2):
                    s_ps = psM.tile([128, N], F32, tag="m")
                    sl = slice(128 * c, 128 * (c + 1))
                    nc.tensor.matmul(out=s_ps, lhsT=m2rt[:, sl], rhs=rt,
                                     start=True, stop=False)
                    nc.tensor.matmul(out=s_ps, lhsT=n_sb[:, sl], rhs=ones_row,
                                     start=False, stop=False)
                    nc.tensor.matmul(out=s_ps, lhsT=ones_row[:, sl], rhs=n_sb,
                                     start=False, stop=True)
                    smax = prep.tile([128, N], F32)
                    nc.vector.tensor_scalar_max(smax, s_ps, 1e-30)
                    lns = prep.tile([128, N], F32)
                    nc.scalar.activation(lns, smax, AF.Ln)
                    dc = dpool.tile([128, N], F32)
                    nc.scalar.activation(dc, lns, AF.Exp, scale=0.5)
                    d_sb.append(dc)
                return d_sb, f_sb

            def main(b, d_sb, f_sb):
                # Per superblock of 8 groups (64 j's): basis for 4 groups via
                # PE selector-matmul, 4 via DMA row-replication (parallel
                # engines); f-broadcast via two K=16 selector matmuls; one
                # K=128 C-matmul per group accumulates into fbar.
                fbar = psF.tile([64, N], F32)
                nmm = [0]
                for s in range(NGRP // 8):
                    # --- basis: groups 8s..8s+3 on PE, 8s+4..8s+7 via DMA ---
                    Pp = psP.tile([128, 4 * N], F32)
                    for gg in range(4):
                        g = s * 8 + gg
                        c = g // (NGRP // 2)
                        G = (g % (NGRP // 2)) // TPB
                        t = g % TPB
                        nc.tensor.matmul(
                            out=Pp[:, N * gg: N * (gg + 1)],
                            lhsT=ksel_sb[32 * G: 32 * (G + 1),
                                         128 * t: 128 * (t + 1)],
                            rhs=d_sb[c][32 * G: 32 * (G + 1), :],
                            start=True, stop=True,
                            tile_position=(32 * G, 0),
                        )
                    Ps0 = pbas.tile([128, 4 * N], F32, tag="ps")
                    nc.scalar.activation(Ps0, Pp, AF.Exp)

                    dbc = pbas.tile([128, 4 * N], F32, tag="dbc")
                    for gg in range(4):
                        g = s * 8 + 4 + gg
                        c = g // (NGRP // 2)
                        p0 = PACK * (g % (NGRP // 2))
                        src_ap = d_sb[c][p0: p0 + PACK, :]
                        eng = (nc.sync, nc.gpsimd)[gg % 2]
                        eng.dma_start(
                            out=dbc[:, N * gg: N * (gg + 1)],
                            in_=bass.AP(tensor=src_ap.tensor,
                                        offset=src_ap.offset,
                                        ap=[src_ap.ap[0], [0, KB], [1, N]]),
                        )
                    Ps1 = pbas.tile([128, 4 * N], F32, tag="ps")
                    nc.scalar.activation(Ps1, dbc, AF.Exp, scale=gamv_sb)
                    Ps = (Ps0, Ps1)

                    # --- f-broadcast: one K=16 matmul per parity t ---
                    cf = []
                    for t in range(2):
                        fb_ps = psM.tile([128, 4 * F], F32, tag="m")
                        nc.tensor.matmul(
                            out=fb_ps,
                            lhsT=fsel_sb[:, 128 * t: 128 * (t + 1)],
                            rhs=f_sb[:, 4 * F * s: 4 * F * (s + 1)],
                            start=True, stop=True,
                        )
                        cft = cfp.tile([128, 4 * F], F32)
                        nc.vector.tensor_mul(cft, cstk_sb, fb_ps)
                        cf.append(cft)

                    # --- C-matmuls: group g = 8s + 2k + t ---
                    for k in range(4):
                        for t in range(2):
                            gg = 2 * k + t
                            nc.tensor.matmul(
                                out=fbar,
                                lhsT=cf[t][:, F * k: F * (k + 1)],
                                rhs=Ps[gg // 4][:, N * (gg % 4): N * (gg % 4 + 1)],
                                start=(nmm[0] == 0), stop=(nmm[0] == NGRP - 1),
                            )
                            nmm[0] += 1
                return fbar

            def epilogue(b, fbar):
                # stable softplus x + ln(1+e^-x) (f_bar > 0), transpose, store
                esb = outp.tile([64, N], F32)
                nc.scalar.activation(esb, fbar, AF.Exp, scale=-1.0)
                lsb = outp.tile([64, N], F32)
                nc.scalar.activation(lsb, esb, AF.Ln, bias=1.0)
                osb = outp.tile([64, N], F32)
                nc.vector.tensor_add(osb, fbar, lsb)
                t_sb = outp.tile([128, 2 * F], F32)
                for hh in range(2):
                    t_ps = psM.tile([128, 64], F32, tag="m")
                    nc.tensor.transpose(t_ps, osb[:, 128 * hh: 128 * (hh + 1)],
                                        ident_sb)
                    nc.vector.tensor_copy(t_sb[:, F * hh: F * (hh + 1)], t_ps)
                nc.sync.dma_start(
                    out=bass.AP(tensor=out_h, offset=b * N * F,
                                ap=[[F, 128], [128 * F, 2], [1, F]]),
                    in_=t_sb,
                )

            def emit_body():
                pro = [prologue(b) for b in range(NB)]
                fbars = [main(b, *pro[b]) for b in range(NB)]
                for b in range(NB):
                    epilogue(b, fbars[b])

            if reps > 1:
                with tc.For_i(0, reps, 1):
                    emit_body()
            else:
                emit_body()

    nc.compile()
    return nc


_CACHE: dict = {}


def _get_nc(reps: int = 1):
    if reps not in _CACHE:
        _CACHE[reps] = build_nc(reps)
    return _CACHE[reps]


def make_fsel() -> np.ndarray:
    """[16, 256]: fsel[j', 128t + 16q + k'] = (j' == 8t + q), so a K=16
    matmul against 16 f-rows replicates row 8t+q across the k' partitions."""
    fsel = np.zeros((16, 256), np.float32)
    for t in range(2):
        for q in range(PACK):
            fsel[8 * t + q, 128 * t + KB * q: 128 * t + KB * q + KB] = 1.0
    return fsel


def make_inputs_per_core(r_batch, f_batch, w1, b1, W2, b2):
    r_batch = np.ascontiguousarray(np.asarray(r_batch, np.float32))
    f_batch = np.ascontiguousarray(np.asarray(f_batch, np.float32))
    C = fit_coeffs(np.asarray(w1, np.float64), np.asarray(b1, np.float64),
                   np.asarray(W2, np.float64), np.asarray(b2, np.float64))
    gam = basis_gammas()
    ksel = make_ksel(gam)
    cstk = make_cstack(C)
    ident = np.eye(64, dtype=np.float32)
    gamv = np.ascontiguousarray(np.tile(gam.astype(np.float32), PACK)[:, None])
    in_maps = []
    for c in range(NCORES):
        in_maps.append({
            "r": r_batch[NB * c: NB * (c + 1)],
            "f": f_batch[NB * c: NB * (c + 1)],
            "fsel": make_fsel(),
            "ksel": ksel,
            "cstk": cstk,
            "ident": ident,
            "gamv": gamv,
        })
    return in_maps


def kernel(r_batch, f_batch, w1, b1, W2, b2):
    in_maps = make_inputs_per_core(r_batch, f_batch, w1, b1, W2, b2)
    nc = _get_nc(1)
    res = run_bass_kernel_spmd(nc, in_maps, list(range(NCORES)))
    out = np.concatenate([res.results[c]["out"] for c in range(NCORES)], axis=0)
    return out.astype(np.float32)
